# revision 7
# baseline (speedup 1.0000x reference)
"""LMMD (DSAN local MMD) loss on 8 Trainium2 NeuronCores.

Math (reference):
    X = concat(source, target)                    # [N=4096, D=1024]
    l2[i,j] = max(|x_i|^2 + |x_j|^2 - 2 x_i.x_j, 0)
    bw      = sum(l2) / (N^2 - N) / 4
    K       = sum_q exp(-l2 / (bw * 2^q)),  q = 0..4
    loss    = sum_c v_c^T K v_c / 12,  V = [s_norm; -t_norm]  (rank-12 weights)

Device decomposition (row-sharded, transposed tiles):
    Core c owns i-block I_c = [512c, 512(c+1)).  It computes, for every
    j-tile (128 rows of the full 4096), the gram block G[j, i in I_c] via
    PE matmuls (lhsT = X^T[:, j-block] streamed bf16 weights, rhs = own
    X^T columns, contraction over D), then
        F_q[j,i] = exp(2 c_q G - c_q sq_j)     (c_q = 1/(bw 2^q))
    on ACT (q=4 and q=0 directly; F_3 = F_4^2, F_2 = F_3^2, F_1 = F_2^2 on
    DVE), and reduces with a second PE matmul into PSUM accumulators
        R_q[cls, i] += V[j, cls]^T F_q[j, i].
    Host applies alpha_q[i] = exp(-c_q sq_i) and the final V-contraction:
        loss = 1/12 sum_q sum_i alpha_q[i] sum_cls V[i,cls] R_q[cls,i].
    bw is computed analytically on host: sum(l2) = 2N*sum(sq) - 2|colsum|^2
    (the relu clamp only affects the diagonal at ~1e-10 relative).
"""

import numpy as np
import ml_dtypes

import concourse.bass as bass
from concourse import bacc
import concourse.mybir as mybir
import concourse.tile as tile
from concourse.bass_utils import run_bass_kernel_spmd

B = 2048
D = 1024
C = 12
NCORES = 8
N = 2 * B                 # 4096 total samples
IPC = N // NCORES         # 512 own columns (i) per core
NJT = N // 128            # 32 j-tiles
NKC = D // 128            # 8 contraction chunks
JG = 4                    # j-groups (weight DMA granularity)
JPG = NJT // JG           # 8 j-tiles per group
NQ = 5                    # kernels in the RBF mixture
DELAY = 2                 # j-tiles of software pipelining for weighted MMs

_BUILT = None             # (nc,) cache — program is input-independent


def _build_program():
    fp32 = mybir.dt.float32
    f32r = mybir.dt.float32r
    bf16 = mybir.dt.bfloat16
    Exp = mybir.ActivationFunctionType.Exp

    nc = bacc.Bacc()
    xtb = nc.declare_dram_parameter("xtb", [D, N], bf16, isOutput=False)
    own = nc.declare_dram_parameter("own", [D, IPC], bf16, isOutput=False)
    vt = nc.declare_dram_parameter("vt", [128, NJT * C], f32r, isOutput=False)
    qsq = nc.declare_dram_parameter("qsq", [128, NJT * NQ], fp32, isOutput=False)
    scl = nc.declare_dram_parameter("scl", [128, NQ], fp32, isOutput=False)
    rout = nc.declare_dram_parameter("r_out", [NQ, C, IPC], fp32, isOutput=True)

    with tile.TileContext(nc) as tc:
        with (
            tc.tile_pool(name="singles", bufs=1) as singles,
            tc.tile_pool(name="wpool", bufs=2) as wpool,
            tc.tile_pool(name="epool", bufs=3) as epool,
            tc.tile_pool(name="ostage", bufs=1) as ostage,
            tc.tile_pool(name="gpsum", bufs=3, space="PSUM") as gpsum,
            tc.tile_pool(name="rqpsum", bufs=1, space="PSUM") as rqpsum,
        ):
            own_sb = singles.tile([128, NKC * IPC], bf16)
            for k in range(NKC):
                nc.sync.dma_start(
                    out=own_sb[:, k * IPC : (k + 1) * IPC],
                    in_=own[k * 128 : (k + 1) * 128, :],
                )
            vt_sb = singles.tile([128, NJT * C], f32r)
            nc.sync.dma_start(out=vt_sb[:], in_=vt[:])
            qsq_sb = singles.tile([128, NJT * NQ], fp32)
            nc.sync.dma_start(out=qsq_sb[:], in_=qsq[:])
            scl_sb = singles.tile([128, NQ], fp32)
            nc.sync.dma_start(out=scl_sb[:], in_=scl[:])
            # Stage small tiles through DVE so consumers wait on one
            # semaphore instead of the DMA queue fan-out (walrus caps the
            # per-instruction sync-wait count).
            vt_s = singles.tile([128, NJT * C], f32r)
            nc.vector.tensor_copy(vt_s, vt_sb)
            qsq_s = singles.tile([128, NJT * NQ], fp32)
            nc.vector.tensor_copy(qsq_s, qsq_sb)
            scl_s = singles.tile([128, NQ], fp32)
            nc.vector.tensor_copy(scl_s, scl_sb)
            # Dummy ACT op: absorbs the DVE wait (walrus allows a single
            # sync-wait slot per Activation), so loop Exp ops only ever
            # need the PE wait.
            warm = singles.tile([128, NQ], fp32)
            nc.scalar.activation(warm, scl_s, Exp)

            rq = [rqpsum.tile([C, IPC], fp32, tag=f"rq{q}", name=f"rq{q}") for q in range(NQ)]

            def emit_weighted(jt, es):
                lhs = vt_s[:, jt * C : (jt + 1) * C]
                for q in range(NQ):
                    nc.tensor.matmul(
                        rq[q],
                        lhsT=lhs,
                        rhs=es[q],
                        start=(jt == 0),
                        stop=(jt == NJT - 1),
                    )

            pending = []
            for jg in range(JG):
                w = [wpool.tile([128, JPG * 128], bf16, tag=f"wk{k}", name=f"w{k}") for k in range(NKC)]
                for k in range(NKC):
                    nc.sync.dma_start(
                        out=w[k],
                        in_=xtb[k * 128 : (k + 1) * 128, jg * JPG * 128 : (jg + 1) * JPG * 128],
                    )
                for jl in range(JPG):
                    jt = jg * JPG + jl
                    g = gpsum.tile([128, IPC], fp32)
                    for k in range(NKC):
                        nc.tensor.matmul(
                            g,
                            lhsT=w[k][:, jl * 128 : (jl + 1) * 128],
                            rhs=own_sb[:, k * IPC : (k + 1) * IPC],
                            start=(k == 0),
                            stop=(k == NKC - 1),
                        )
                    e4 = epool.tile([128, IPC], f32r, bufs=NJT)
                    e3 = epool.tile([128, IPC], f32r)
                    e2 = epool.tile([128, IPC], f32r)
                    e1 = epool.tile([128, IPC], f32r)
                    e0 = epool.tile([128, IPC], f32r)
                    nc.scalar.activation(
                        e4, g, Exp,
                        bias=qsq_s[:, jt * NQ + 4 : jt * NQ + 5],
                        scale=scl_s[:, 4:5],
                    )
                    nc.scalar.activation(
                        e0, g, Exp,
                        bias=qsq_s[:, jt * NQ : jt * NQ + 1],
                        scale=scl_s[:, 0:1],
                    )
                    nc.vector.tensor_mul(e3, e4, e4)
                    nc.vector.tensor_mul(e2, e3, e3)
                    nc.vector.tensor_mul(e1, e2, e2)
                    pending.append((jt, [e0, e1, e2, e3, e4]))
                    if len(pending) > DELAY:
                        emit_weighted(*pending.pop(0))
            for item in pending:
                emit_weighted(*item)

            for q in range(NQ):
                stg = ostage.tile([C, IPC], fp32, tag=f"st{q}", name=f"st{q}")
                nc.vector.tensor_copy(stg, rq[q])
                nc.sync.dma_start(out=rout[q], in_=stg)

    nc.compile()
    return nc


def _prep(source, target, source_label, target_logits):
    X = np.concatenate([np.asarray(source), np.asarray(target)], axis=0)
    X64 = X.astype(np.float64)
    sq = np.einsum("nd,nd->n", X64, X64)
    colsum = X64.sum(axis=0)
    sum_l2 = 2.0 * N * sq.sum() - 2.0 * (colsum @ colsum)
    bw = sum_l2 / (N * N - N) / (2.0 ** (5 // 2))
    cq = np.array([1.0 / (bw * 2.0**q) for q in range(NQ)])  # [5]

    sl = np.asarray(source_label, np.float64)
    tl = np.asarray(target_logits, np.float64)
    ssum = sl.sum(0)
    s_norm = np.where(ssum > 0, sl / np.where(ssum > 0, ssum, 1.0), 0.0)
    tsum = tl.sum(0)
    t_norm = np.where(tsum > 0, tl / np.where(tsum > 0, tsum, 1.0), 0.0)
    s_pres = np.zeros(C)
    np.add.at(s_pres, sl.argmax(1), 1.0)
    t_pres = np.zeros(C)
    np.add.at(t_pres, tl.argmax(1), 1.0)
    common = ((s_pres > 0) & (t_pres > 0)).astype(np.float64)
    V = np.concatenate([s_norm * common, -t_norm * common], axis=0)  # [N, C]

    xtb = np.ascontiguousarray(X.T).astype(ml_dtypes.bfloat16)  # [D, N]
    vt = np.ascontiguousarray(
        V.reshape(NJT, 128, C).transpose(1, 0, 2).reshape(128, NJT * C)
    ).astype(np.float32)
    # qsq[p, jt*5+q] = -c_q * sq[jt*128 + p]
    sqt = sq.reshape(NJT, 128)
    qsq = np.ascontiguousarray(
        (-cq[None, None, :] * sqt[:, :, None]).transpose(1, 0, 2).reshape(128, NJT * NQ)
    ).astype(np.float32)
    scl = np.broadcast_to((2.0 * cq).astype(np.float32), (128, NQ)).copy()
    return X, sq, cq, V, xtb, vt, qsq, scl


def _postprocess(results, sq, cq, V):
    # loss = 1/12 sum_q sum_i alpha_q[i] * (sum_cls V[i,cls] R_q[cls,i])
    loss = 0.0
    for c in range(NCORES):
        r = np.asarray(results[c]["r_out"], np.float64)  # [5, 12, 512]
        i0 = c * IPC
        Vc = V[i0 : i0 + IPC]                  # [512, 12]
        alpha = np.exp(-np.outer(cq, sq[i0 : i0 + IPC]))  # [5, 512]
        loss += np.einsum("qi,ic,qci->", alpha, Vc, r)
    return loss / C


def _run(in_maps, trace=False, **kw):
    global _BUILT
    if _BUILT is None:
        _BUILT = _build_program()
    return run_bass_kernel_spmd(_BUILT, in_maps, list(range(NCORES)), trace=trace, **kw)


def kernel(source, target, source_label, target_logits, _trace=False, _ret_bkr=False):
    X, sq, cq, V, xtb, vt, qsq, scl = _prep(source, target, source_label, target_logits)
    in_maps = []
    for c in range(NCORES):
        own = np.ascontiguousarray(xtb[:, c * IPC : (c + 1) * IPC])
        in_maps.append(
            {"xtb": xtb, "own": own, "vt": vt, "qsq": qsq, "scl": scl}
        )
    bkr = _run(in_maps, trace=_trace)
    loss = _postprocess(bkr.results, sq, cq, V)
    out = np.float32(loss)
    if _ret_bkr:
        return out, bkr
    return out
